# revision 40
# baseline (speedup 1.0000x reference)
"""NT-Xent / SimCLR contrastive loss on 8 Trainium2 NeuronCores (Bass/Tile).

Problem: zi, zj [4096, 512] f32 -> scalar loss.
  reps = concat(zi, zj)            [8192, 512]
  rn   = reps / max(||reps||, 1e-8)
  sim  = rn @ rn.T                 [8192, 8192]
  pos_i  = sim[i, (i+B) mod 2B]
  denom_i = sum_{j != i} exp(sim_ij / tau)
  loss = mean(-pos/tau + log(denom))

Sharding (per the hint, each device starts from its shard of the
normalized reps): the host normalizes + shards (the "each device holds
its row block of normalized reps" initial state), each core computes its
block-symmetric share of the similarity GEMM and the exp/partial-sum
reductions, and the host performs the final cross-core assembly + log +
mean (the scalar all-reduce).

Symmetric decomposition: core c owns rows [c*1024, (c+1)*1024) and
computes
  - its rows x blocks {c+1, c+2, c+3} (mod 8), 3072 cols, in full;
  - the own block's upper triangle only (cols >= t*128 for m-tile t) -
    the strictly-lower part is recovered on the host from the own-block
    column sums by symmetry;
  - two antipodal quadrants vs block b=(c+4)%8 (512 cols per row half).
Row sums of exp come from the ACT accumulator; column sums of exp (the
mirrored row-partials destined for other rows) are accumulated on the
vector engine into a [128, 5120] buffer and shipped to the host, which
folds the 128 partitions, assembles denom from all partials, and takes
log + mean.

Speed: operands are fp8e4 (16*rn, exact power-of-2 scale) so the PE runs
DoubleRow perf mode (K=256 per pass; on real TRN2 this streams ~1
column/cycle at the power-throttled ~1.2-1.6GHz, ~1.8x faster than
bf16); exp tiles are [128, 2048] (4 PSUM banks) to amortize ACT access +
accumulator-read overhead; the 8 antipodal quadrants are packed
4-per-PSUM-tile so all 8 cost only two ACT instructions.  The ACT exp is
the ONLY PSUM reader (critical: any DVE read of PSUM gates the PE's
PSUM-slot reuse on the DVE queue): the self-exclusion term and the
positives are extracted post-exp from the SBUF exp tiles.  The T0 tile
is f32 so the extracted diagonal equals the accumulated self term
bit-exactly and rowsum - selfexp cancels exactly (selfexp ~ e^{1/tau} ~
1.6e6 vs denom ~ 1e4); positives come back as exp(pos/tau) and the host
takes log.  Redundant Ldweights (bass emits one per matmul) are pruned
post-compile.  Input pieces stream over all three DMA queues (SP + ACT
HWDGE, GpSimd SWDGE) in consumption order so the GEMM chases the DMA
wavefront.
"""

import sys

for _p in ("/opt/trn_rl_repo",):
    if _p not in sys.path:
        sys.path.insert(0, _p)

from contextlib import ExitStack

import ml_dtypes
import numpy as np

TAU = 0.07
B, D = 4096, 512
NCORES = 8
ROWS = 2 * B              # 8192
RPC = ROWS // NCORES      # 1024 rows per core
NM = RPC // 128           # 8 m-tiles per core
KC = D // 128             # 4 k-subtiles of 128
CTOT = 5 * RPC            # 5120 GEMM columns per core
NA = 4 * RPC              # 4096 GEMM-A columns
CJ = 5120                 # colacc width: [g1|g2|g3|quads|own-tri]
FP8S = 16.0               # operand scale (power of 2, exact in fp8)
SCALE = 1.0 / (FP8S * FP8S * TAU)   # ACT exp scale on PSUM values

_prog_cache = {}


def _build_program():
    import concourse.bacc as bacc
    import concourse.tile as tile
    import concourse.mybir as mybir
    import bass_rust

    dt = mybir.dt
    Alu = mybir.AluOpType
    Act = mybir.ActivationFunctionType
    DR = mybir.MatmulPerfMode.DoubleRow

    nc = bacc.Bacc("TRN2", target_bir_lowering=False, debug=False,
                   enable_asserts=False, num_devices=NCORES)

    rnT_in = nc.dram_tensor("rnT", [128, KC, CTOT], dt.float8e4,
                            kind="ExternalInput").ap()
    ident_f32 = nc.dram_tensor("ident_f32", [128, 128], dt.float32,
                               kind="ExternalInput").ap()
    out = nc.dram_tensor("out", [128, 16], dt.float32,
                         kind="ExternalOutput").ap()
    cacc_out = nc.dram_tensor("cacc_out", [128, CJ], dt.bfloat16,
                              kind="ExternalOutput").ap()
    e1last = nc.dram_tensor("e1last", [128, 2048], dt.bfloat16,
                            kind="ExternalOutput").ap()

    with tile.TileContext(nc) as tc, ExitStack() as ctx:
        const = ctx.enter_context(tc.tile_pool(name="const", bufs=1))
        persist = ctx.enter_context(tc.tile_pool(name="persist", bufs=1))
        e0p = ctx.enter_context(tc.tile_pool(name="e0p", bufs=3))
        e1p = ctx.enter_context(tc.tile_pool(name="e1p", bufs=3))
        scrp = ctx.enter_context(tc.tile_pool(name="scrp", bufs=3))
        smallp = ctx.enter_context(tc.tile_pool(name="smallp", bufs=4))
        ps = ctx.enter_context(tc.tile_pool(name="ps", bufs=2,
                                            space="PSUM"))

        i32 = const.tile([128, 128], dt.float32, tag="i32")

        # Hoist the ACT table load off the critical path: a throwaway
        # activation at the top of the ACT program makes bacc place the
        # (1.3us) table load in the startup window instead of in front of
        # the first real exp.
        warm_in = smallp.tile([128, 8], dt.float32, tag="warm_in")
        warm_out = smallp.tile([128, 8], dt.float32, tag="warm_out")
        nc.vector.tensor_scalar(out=warm_in[:], in0=warm_in[:], scalar1=0.0,
                                scalar2=None, op0=Alu.mult)
        nc.scalar.activation(warm_out[:], warm_in[:], Act.Exp, scale=SCALE)

        # rnT is split into one tile per 256KB piece (k-subtile-pair,
        # 512-col window-half) so Tile's per-tile write tracking lets each
        # Ldweights/matmul start as soon as ITS piece lands, and the
        # pieces round-robin across all three DMA queues (SP + ACT HWDGE,
        # GpSimd SWDGE) in consumption order: the input load is
        # HBM-bandwidth-bound (~8.5us for 2.6MB), so the GEMM chases the
        # DMA wavefront instead of waiting for it to finish.
        rnP = {}
        for cp in range(2):
            for w in range(5):
                for h in range(2):
                    tl = persist.tile([128, 2 * 512], dt.float8e4,
                                      tag=f"rn{cp}{w}{h}")
                    rnP[(cp, w, h)] = tl[:].rearrange("p (c w) -> p c w",
                                                      c=2)
        colacc = persist.tile([128, CJ], dt.bfloat16, tag="colacc")
        rs = persist.tile([128, 16], dt.float32, tag="rs")
        sea = persist.tile([128, NM], dt.float32, tag="sea")
        qsum = persist.tile([128, NM], dt.float32, tag="qsum")
        outbuf = persist.tile([128, 16], dt.float32, tag="outbuf")

        # Explicit per-queue piece lists, ordered by consumption time:
        # T0(m0) = windows 0+1 both k-pairs (first 1MB) lands first so the
        # ACT stream starts ~6us earlier; the Scalar queue carries only
        # late-needed pieces (its head hosts the ACT table load).
        dma_in = {}
        _qlists = [
            (nc.sync, [(0, 0, 0), (0, 0, 1), (0, 1, 0), (0, 1, 1),
                       (1, 0, 0), (1, 1, 1), (1, 3, 0), (0, 4, 0),
                       (1, 4, 0)]),
            (nc.gpsimd, [(1, 0, 1), (1, 1, 0), (0, 2, 0), (0, 3, 0),
                         (1, 2, 0), (0, 4, 1), (1, 4, 1)]),
            (nc.scalar, [(0, 2, 1), (0, 3, 1), (1, 2, 1), (1, 3, 1)]),
        ]
        nc.scalar.dma_start(i32[:], ident_f32[:])
        for q, lst in _qlists:
            for cp, w, h in lst:
                dma_in[(cp, w, h)] = q.dma_start(
                    rnP[(cp, w, h)],
                    rnT_in[:, 2 * cp:2 * cp + 2,
                           w * 1024 + h * 512:w * 1024 + (h + 1) * 512])

        def sdep(inst, dma):
            inst.ins.add_dependency(dma.ins.name,
                                    bass_rust.DependencyInfo.SYNC_ONLY)

        # PSUM WAR edges are missing from Tile's tracker: a slot-recycling
        # matmul (start=True resets the region) must wait for the previous
        # occupant's readers (exp / diag STT). Track readers per pool slot.
        ps_readers = {}
        mv_dep_done = set()

        def war_dep(mm, readers):
            for rname in readers:
                mm.ins.add_dependency(rname, bass_rust.DependencyInfo.SYNC_ONLY)

        # The matmul MOVING-operand read has no tracked edge to the DMA
        # that writes it (only the Ldweights/stationary read is tracked);
        # add one manual edge per piece on its first reader.
        def mv_dep(mm, cp, w, h):
            if (cp, w, h) not in mv_dep_done:
                mv_dep_done.add((cp, w, h))
                sdep(mm, dma_in[(cp, w, h)])

        def stat_ap(c2, t):
            return rnP[(c2, 0, t // 4)][:, :, (t % 4) * 128:
                                        (t % 4) * 128 + 128]

        tilectr = 0
        selfexp_t = []

        def gemm_a_tile(t, last=False):
            """m-tile t.

            T0 PSUM layout: [g1 (1024) | own-triangle (1024-128t)], the
            own block computed only for cols >= t*128 (the strictly-lower
            part is recovered on the host from the own colacc region by
            symmetry).  The triangle splits into 512-aligned PSUM pieces
            so matmul outputs never straddle a bank: t<4 -> full (w0,h1)
            piece at 1024 + partial (w0,h0) at 1536; t>=4 -> partial
            (w0,h1) at 1024.  The own-diag block is the partial piece's
            head.  T1 = [+2|+3] unchanged.
            """
            nonlocal tilectr
            slot0 = tilectr % 2
            slot1 = (tilectr + 1) % 2
            ps0 = ps.tile([128, 2048], dt.float32, tag="ps")
            ps1 = ps.tile([128, 2048], dt.float32, tag="ps")
            w0 = 2048 - 128 * t       # T0 width
            if t < 4:
                tri = [(0, 128 * t, 512 - 128 * t, 1536), (1, 0, 512, 1024)]
                dcol = 1536
            else:
                tri = [(1, 128 * (t - 4), 1024 - 128 * t, 1024)]
                dcol = 1024
            for c2 in range(2):
                stat = stat_ap(c2, t)
                # T0: triangle pieces (window 0, lands first) then g1
                t0_pieces = ([(0, h, off, ln, pcol)
                              for (h, off, ln, pcol) in tri]
                             + [(1, h, 0, 512, h * 512) for h in range(2)])

                def t0_mms():
                    for pi, (w, h, off, ln, pcol) in enumerate(t0_pieces):
                        mm = nc.tensor.matmul(
                            ps0[:, pcol:pcol + ln], stat,
                            rnP[(c2, w, h)][:, :, off:off + ln],
                            start=(c2 == 0), stop=(c2 == 1), perf_mode=DR)
                        if c2 == 0 and pi == 0:
                            war_dep(mm, ps_readers.get(slot0, ()))
                        mv_dep(mm, c2, w, h)

                def t1_mms():
                    for piece in range(4):
                        w = 2 + piece // 2
                        h = piece % 2
                        mm = nc.tensor.matmul(
                            ps1[:, piece * 512:(piece + 1) * 512], stat,
                            rnP[(c2, w, h)][:, :, :],
                            start=(c2 == 0), stop=(c2 == 1), perf_mode=DR)
                        if c2 == 0 and piece == 0:
                            war_dep(mm, ps_readers.get(slot1, ()))
                        mv_dep(mm, c2, w, h)

                if last and c2 == 1:
                    # finish T1 first so its (larger) exp + raw ship
                    # overlap T0's final matmuls
                    t1_mms()
                    t0_mms()
                else:
                    t0_mms()
                    t1_mms()
            if last:
                # T1's exp + raw ship go FIRST (T1 matmuls finished first
                # above); they overlap T0's final matmuls + exp
                e1 = e1p.tile([128, 2048], dt.bfloat16, tag="e1")
                ex1 = nc.scalar.activation(e1[:], ps1[:], Act.Exp,
                                           scale=SCALE,
                                           accum_out=rs[:, 8 + t:9 + t])
                ps_readers[slot1] = [ex1.ins.name]
                nc.sync.dma_start(e1last[:], e1[:])
            # T0 exp (f32: the stored diag equals the accumulated self
            # term bit-exactly, keeping the self-cancellation exact)
            e0 = e0p.tile([128, 2048], dt.float32, tag="e0")
            ex0 = nc.scalar.activation(e0[:, 0:w0], ps0[:, 0:w0], Act.Exp,
                                       scale=SCALE,
                                       accum_out=rs[:, t:t + 1])
            ps_readers[slot0] = [ex0.ins.name]
            # self-exclusion: extract exp(self/tau) from the SBUF exp tile
            # (not PSUM - the exp is then the only PSUM reader and the PE
            # is never gated on the DVE)
            scr = scrp.tile([128, 128], dt.float32, tag="scrd")
            nc.vector.scalar_tensor_tensor(
                out=scr[:], in0=e0[:, dcol:dcol + 128], scalar=1.0,
                in1=i32[:], op0=Alu.mult, op1=Alu.mult,
                accum_out=sea[:, t:t + 1])
            # colacc block +1 (cols 1024..2048 -> colacc 0..1024)
            if t == 0:
                nc.vector.tensor_scalar(out=colacc[:, 0:1024],
                                        in0=e0[:, 0:1024], scalar1=0.0,
                                        scalar2=None, op0=Alu.add)
            else:
                nc.vector.tensor_add(colacc[:, 0:1024], colacc[:, 0:1024],
                                     e0[:, 0:1024])
            # own-triangle colacc (diag block excluded): region 4096..5120
            if t < 4:
                if t == 0:
                    nc.vector.tensor_scalar(out=colacc[:, 4608:5120],
                                            in0=e0[:, 1024:1536],
                                            scalar1=0.0, scalar2=None,
                                            op0=Alu.add)
                else:
                    nc.vector.tensor_add(colacc[:, 4608:5120],
                                         colacc[:, 4608:5120],
                                         e0[:, 1024:1536])
                ln2 = 384 - 128 * t   # partial piece minus diag block
                if ln2 > 0:
                    lo = 4224 + 128 * t
                    if t == 0:
                        nc.vector.tensor_scalar(out=colacc[:, lo:4608],
                                                in0=e0[:, 1664:1664 + ln2],
                                                scalar1=0.0, scalar2=None,
                                                op0=Alu.add)
                    else:
                        nc.vector.tensor_add(colacc[:, lo:4608],
                                             colacc[:, lo:4608],
                                             e0[:, 1664:1664 + ln2])
            else:
                ln2 = 896 - 128 * t
                if ln2 > 0:
                    lo = 4224 + 128 * t
                    nc.vector.tensor_add(colacc[:, lo:lo + ln2],
                                         colacc[:, lo:lo + ln2],
                                         e0[:, 1152:1152 + ln2])
            if last:
                # +1 and own colacc regions final
                nc.scalar.dma_start(cacc_out[:, 0:1024], colacc[:, 0:1024])
                nc.scalar.dma_start(cacc_out[:, 4096:5120],
                                    colacc[:, 4096:5120])
            else:
                # T1: bf16 exp, colacc blocks +2/+3
                e1 = e1p.tile([128, 2048], dt.bfloat16, tag="e1")
                ex1 = nc.scalar.activation(e1[:], ps1[:], Act.Exp,
                                           scale=SCALE,
                                           accum_out=rs[:, 8 + t:9 + t])
                ps_readers[slot1] = [ex1.ins.name]
                if t == 0:
                    nc.vector.tensor_scalar(out=colacc[:, 1024:3072],
                                            in0=e1[:], scalar1=0.0,
                                            scalar2=None, op0=Alu.add)
                else:
                    nc.vector.tensor_add(colacc[:, 1024:3072],
                                         colacc[:, 1024:3072], e1[:])
            tilectr += 2

        def quad_tile(qt):
            """antipodal quadrants for m-tiles 4qt..4qt+3, packed in one
            PSUM tile; row sums on the DVE, colacc region final after."""
            nonlocal tilectr
            slot = tilectr % 2
            psq = ps.tile([128, 2048], dt.float32, tag="ps")
            # fill + exp per 2-quad half so the ACT stream restarts
            # ~2us earlier at each quad transition
            eq = e1p.tile([128, 2048], dt.bfloat16, tag="e1")
            readers = []
            for half in range(2):
                for c2 in range(2):
                    for i in (half * 2, half * 2 + 1):
                        t = qt * 4 + i
                        h = 0 if t < 4 else 1
                        mm = nc.tensor.matmul(
                            psq[:, i * 512:(i + 1) * 512], stat_ap(c2, t),
                            rnP[(c2, 4, h)][:, :, :],
                            start=(c2 == 0), stop=(c2 == 1), perf_mode=DR)
                        if half == 0 and c2 == 0 and i == 0:
                            war_dep(mm, ps_readers.get(slot, ()))
                        mv_dep(mm, c2, 4, h)
                hs = slice(half * 1024, (half + 1) * 1024)
                exq = nc.scalar.activation(eq[:, hs], psq[:, hs], Act.Exp,
                                           scale=SCALE)
                readers.append(exq.ins.name)
            ps_readers[slot] = readers
            # positives: post-exp quadrant diagonals (host takes log)
            for i in range(4):
                t = qt * 4 + i
                scr = scrp.tile([128, 128], dt.float32, tag="scrd")
                nc.vector.scalar_tensor_tensor(
                    out=scr[:], in0=eq[:, i * 512 + (t % 4) * 128:
                                       i * 512 + (t % 4) * 128 + 128],
                    scalar=1.0, in1=i32[:], op0=Alu.mult, op1=Alu.mult,
                    accum_out=outbuf[:, 8 + t:9 + t])
            # row sums of the 4 quads on the DVE (one op)
            nc.vector.reduce_sum(qsum[:, qt * 4:qt * 4 + 4],
                                 eq[:].rearrange("p (a w) -> p a w", a=4),
                                 axis=mybir.AxisListType.X)
            # colacc quad region 3072+qt*512 .. 3584+qt*512
            creg = slice(3072 + qt * 512, 3584 + qt * 512)
            for i in range(4):
                esub = eq[:, i * 512:(i + 1) * 512]
                if i == 0:
                    nc.vector.tensor_scalar(out=colacc[:, creg], in0=esub,
                                            scalar1=0.0, scalar2=None,
                                            op0=Alu.add)
                else:
                    nc.vector.tensor_add(colacc[:, creg], colacc[:, creg],
                                         esub)
            tilectr += 1
            # quad colacc region is final: stream it out early
            nc.gpsimd.dma_start(cacc_out[:, creg], colacc[:, creg])

        # Quad phases interleave mid-GEMM so their exp/colacc/reduce work
        # and output DMAs overlap GEMM-A instead of forming a tail.
        for t in (0, 1, 2, 3):
            gemm_a_tile(t)
        quad_tile(0)
        for t in (4, 5, 6):
            gemm_a_tile(t)
        # +2/+3 colacc region is final after m6 (m7's T1 ships raw);
        # sync queue is idle mid-stream
        nc.sync.dma_start(cacc_out[:, 1024:2048], colacc[:, 1024:2048])
        nc.sync.dma_start(cacc_out[:, 2048:3072], colacc[:, 2048:3072])
        quad_tile(1)
        gemm_a_tile(7, last=True)

        # ---- epilogue ----
        rsum = smallp.tile([128, NM], dt.float32, tag="rsum")
        nc.vector.tensor_add(rsum[:], rs[:, 0:8], rs[:, 8:16])
        nc.vector.tensor_add(rsum[:], rsum[:], qsum[:])
        nc.vector.tensor_sub(outbuf[:, 0:8], rsum[:], sea[:])
        nc.sync.dma_start(out[:], outbuf[:])

    # Pin bacc's activation-table choice to the one table holding Exp (and
    # Ln/Copy) so exactly one ACT table load is emitted.
    import concourse.bacc as bacc_mod
    _orig_tables = bacc_mod.get_activation_tables

    def _only_lnexp(arch):
        keep = "natural_log_exp_and_others"
        return {k: (v if k == keep else set())
                for k, v in _orig_tables(arch).items()}

    bacc_mod.get_activation_tables = _only_lnexp
    try:
        nc.compile()
    finally:
        bacc_mod.get_activation_tables = _orig_tables
    _prune_redundant_ldweights(nc)
    return nc


def _prune_redundant_ldweights(nc):
    """Drop InstLdweights that reload the stationary already resident in
    the PE array (bass emits one per matmul; the array keeps weights
    across matmuls, so only the first load of each group is needed).
    Only sync-free loads (no semaphore waits/updates) with a weights AP
    identical to the previously kept load are removed: ~100 of 136 go,
    each ~150-230ns of PE pipeline."""
    for f in nc.m.functions:
        for b in f.blocks:
            keep = []
            last_sig = None
            changed = False
            for ins in b.instructions:
                tn = type(ins).__name__
                if tn == 'InstLdweights':
                    sig = (str(ins.ins[0]), str(ins.perf_mode),
                           str(ins.is_transpose))
                    clean = (ins.sync_info is None
                             and not ins.has_wait()
                             and not ins.has_update())
                    if clean and sig == last_sig:
                        changed = True
                        continue
                    last_sig = sig
                keep.append(ins)
            if changed:
                b.instructions = keep


def _col_rows(c):
    """Global row indices of core c's 5120 GEMM columns, in rnT order."""
    b = (c + 4) % NCORES
    idxs = [np.arange(((c + d) % NCORES) * RPC, ((c + d) % NCORES + 1) * RPC)
            for d in range(4)]
    if c < 4:
        q = np.arange(b * RPC, (b + 1) * RPC)
    else:
        q = np.concatenate([np.arange(b * RPC + 512, (b + 1) * RPC),
                            np.arange(b * RPC, b * RPC + 512)])
    idxs.append(q)
    return np.concatenate(idxs)


def _host_inputs(zi, zj):
    reps = np.concatenate([np.asarray(zi, np.float64),
                           np.asarray(zj, np.float64)], axis=0)
    norms = np.maximum(np.linalg.norm(reps, axis=1, keepdims=True), 1e-8)
    rn8 = (FP8S * reps / norms).astype(np.float32).astype(
        ml_dtypes.float8_e4m3)                              # [8192, 512]
    ident_f32 = np.eye(128, dtype=np.float32)
    in_maps = []
    for c in range(NCORES):
        xt = rn8[_col_rows(c)].T                            # [512, 5120]
        rnT = np.ascontiguousarray(
            xt.reshape(KC, 128, CTOT).transpose(1, 0, 2))   # [128, 4, 5120]
        in_maps.append({"rnT": rnT, "ident_f32": ident_f32})
    return in_maps


def _postprocess(results):
    denom = np.zeros(ROWS, np.float64)
    pos = np.zeros(ROWS, np.float64)
    for c in range(NCORES):
        o = np.asarray(results[c]["out"], np.float64)        # [128, 16]
        ca = np.asarray(results[c]["cacc_out"], np.float64)  # [128, 5120]
        e1l = np.asarray(results[c]["e1last"], np.float64)   # [128, 2048]
        cr = _col_rows(c)
        for t in range(NM):
            rows = slice(c * RPC + t * 128, c * RPC + (t + 1) * 128)
            denom[rows] += o[:, t]
        # colsum partials: fold partitions, scatter to owning rows.
        # [0:3072] = blocks +1/+2/+3 (m7's +2/+3 tile shipped raw in
        # e1last), [3072:4096] = antipodal quadrants, [4096:5120] = the
        # own-block triangle (strictly-lower part of own rows by
        # symmetry; rows 0..128 of the block have no lower part).
        colsum = ca.sum(axis=0)                              # [5120]
        np.add.at(denom, cr[1024:4096], colsum[0:3072])
        np.add.at(denom, cr[4096:5120], colsum[3072:4096])
        np.add.at(denom, cr[2048:4096], e1l.sum(axis=0))
        np.add.at(denom, cr[128:1024], colsum[4224:5120])
        if c < 4:
            # o[:, 8:16] = exp(pos/tau) (post-exp diag extraction)
            opos = o[:, 8:16].T.reshape(-1)                  # [1024]
            rows = np.arange(c * RPC, (c + 1) * RPC)
            pos[rows] = opos
            pos[cr[4096:]] = opos
    loss = np.mean(-np.log(pos) + np.log(denom))
    return np.asarray(loss, dtype=np.float32)


def kernel(zi, zj, _trace=False):
    from concourse.bass_utils import run_bass_kernel_spmd

    if "nc" not in _prog_cache:
        _prog_cache["nc"] = _build_program()
    nc = _prog_cache["nc"]
    in_maps = _host_inputs(zi, zj)
    res = run_bass_kernel_spmd(nc, in_maps, list(range(NCORES)),
                               trace=_trace)
    _prog_cache["last_result"] = res
    return _postprocess(res.results)


# revision 42
# speedup vs baseline: 1.0196x; 1.0196x over previous
"""NT-Xent / SimCLR contrastive loss on 8 Trainium2 NeuronCores (Bass/Tile).

Problem: zi, zj [4096, 512] f32 -> scalar loss.
  reps = concat(zi, zj)            [8192, 512]
  rn   = reps / max(||reps||, 1e-8)
  sim  = rn @ rn.T                 [8192, 8192]
  pos_i  = sim[i, (i+B) mod 2B]
  denom_i = sum_{j != i} exp(sim_ij / tau)
  loss = mean(-pos/tau + log(denom))

Sharding (per the hint, each device starts from its shard of the
normalized reps): the host normalizes + shards (the "each device holds
its row block of normalized reps" initial state), each core computes its
block-symmetric share of the similarity GEMM and the exp/partial-sum
reductions, and the host performs the final cross-core assembly + log +
mean (the scalar all-reduce).

Symmetric decomposition: core c owns rows [c*1024, (c+1)*1024) and
computes
  - its rows x blocks {c+1, c+2, c+3} (mod 8), 3072 cols, in full;
  - the own block's upper triangle only (cols >= t*128 for m-tile t) -
    the strictly-lower part is recovered on the host from the own-block
    column sums by symmetry;
  - two antipodal quadrants vs block b=(c+4)%8 (512 cols per row half).
Row sums of exp come from the ACT accumulator; column sums of exp (the
mirrored row-partials destined for other rows) are accumulated on the
vector engine into a [128, 5120] buffer and shipped to the host, which
folds the 128 partitions, assembles denom from all partials, and takes
log + mean.

Speed: operands are fp8e4 (16*rn, exact power-of-2 scale) so the PE runs
DoubleRow perf mode (K=256 per pass; on real TRN2 this streams ~1
column/cycle at the power-throttled ~1.2-1.6GHz, ~1.8x faster than
bf16); exp tiles are [128, 2048] (4 PSUM banks) to amortize ACT access +
accumulator-read overhead; the 8 antipodal quadrants are packed
4-per-PSUM-tile so all 8 cost only two ACT instructions.  The ACT exp is
the ONLY PSUM reader (critical: any DVE read of PSUM gates the PE's
PSUM-slot reuse on the DVE queue): the self-exclusion term and the
positives are extracted post-exp from the SBUF exp tiles.  The T0 tile
is f32 so the extracted diagonal equals the accumulated self term
bit-exactly and rowsum - selfexp cancels exactly (selfexp ~ e^{1/tau} ~
1.6e6 vs denom ~ 1e4); positives come back as exp(pos/tau) and the host
takes log.  Redundant Ldweights (bass emits one per matmul) are pruned
post-compile.  Input pieces stream over all three DMA queues (SP + ACT
HWDGE, GpSimd SWDGE) in consumption order so the GEMM chases the DMA
wavefront.
"""

import sys

for _p in ("/opt/trn_rl_repo",):
    if _p not in sys.path:
        sys.path.insert(0, _p)

from contextlib import ExitStack

import ml_dtypes
import numpy as np

TAU = 0.07
B, D = 4096, 512
NCORES = 8
ROWS = 2 * B              # 8192
RPC = ROWS // NCORES      # 1024 rows per core
NM = RPC // 128           # 8 m-tiles per core
KC = D // 128             # 4 k-subtiles of 128
CTOT = 5 * RPC            # 5120 GEMM columns per core
NA = 4 * RPC              # 4096 GEMM-A columns
CJ = 5120                 # colacc width: [g1|g2|g3|quads|own-tri]
FP8S = 16.0               # operand scale (power of 2, exact in fp8)
SCALE = 1.0 / (FP8S * FP8S * TAU)   # ACT exp scale on PSUM values

_prog_cache = {}


def _build_program():
    import concourse.bacc as bacc
    import concourse.tile as tile
    import concourse.mybir as mybir
    import bass_rust

    dt = mybir.dt
    Alu = mybir.AluOpType
    Act = mybir.ActivationFunctionType
    DR = mybir.MatmulPerfMode.DoubleRow

    nc = bacc.Bacc("TRN2", target_bir_lowering=False, debug=False,
                   enable_asserts=False, num_devices=NCORES)

    rnT_in = nc.dram_tensor("rnT", [128, KC, CTOT], dt.float8e4,
                            kind="ExternalInput").ap()
    ident_f32 = nc.dram_tensor("ident_f32", [128, 128], dt.float32,
                               kind="ExternalInput").ap()
    out = nc.dram_tensor("out", [128, 16], dt.float32,
                         kind="ExternalOutput").ap()
    cacc_out = nc.dram_tensor("cacc_out", [128, CJ], dt.bfloat16,
                              kind="ExternalOutput").ap()
    e1last = nc.dram_tensor("e1last", [128, 2048], dt.bfloat16,
                            kind="ExternalOutput").ap()

    with tile.TileContext(nc) as tc, ExitStack() as ctx:
        const = ctx.enter_context(tc.tile_pool(name="const", bufs=1))
        persist = ctx.enter_context(tc.tile_pool(name="persist", bufs=1))
        e0p = ctx.enter_context(tc.tile_pool(name="e0p", bufs=4))
        e1p = ctx.enter_context(tc.tile_pool(name="e1p", bufs=4))
        scrp = ctx.enter_context(tc.tile_pool(name="scrp", bufs=3))
        smallp = ctx.enter_context(tc.tile_pool(name="smallp", bufs=4))
        ps = ctx.enter_context(tc.tile_pool(name="ps", bufs=2,
                                            space="PSUM"))

        i32 = const.tile([128, 128], dt.float32, tag="i32")

        # Hoist the ACT table load off the critical path: a throwaway
        # activation at the top of the ACT program makes bacc place the
        # (1.3us) table load in the startup window instead of in front of
        # the first real exp.
        warm_in = smallp.tile([128, 8], dt.float32, tag="warm_in")
        warm_out = smallp.tile([128, 8], dt.float32, tag="warm_out")
        nc.vector.tensor_scalar(out=warm_in[:], in0=warm_in[:], scalar1=0.0,
                                scalar2=None, op0=Alu.mult)
        nc.scalar.activation(warm_out[:], warm_in[:], Act.Exp, scale=SCALE)

        # rnT is split into one tile per 256KB piece (k-subtile-pair,
        # 512-col window-half) so Tile's per-tile write tracking lets each
        # Ldweights/matmul start as soon as ITS piece lands, and the
        # pieces round-robin across all three DMA queues (SP + ACT HWDGE,
        # GpSimd SWDGE) in consumption order: the input load is
        # HBM-bandwidth-bound (~8.5us for 2.6MB), so the GEMM chases the
        # DMA wavefront instead of waiting for it to finish.
        rnP = {}
        for cp in range(2):
            for w in range(5):
                for h in range(2):
                    tl = persist.tile([128, 2 * 512], dt.float8e4,
                                      tag=f"rn{cp}{w}{h}")
                    rnP[(cp, w, h)] = tl[:].rearrange("p (c w) -> p c w",
                                                      c=2)
        colacc = persist.tile([128, CJ], dt.bfloat16, tag="colacc")
        rs = persist.tile([128, 16], dt.float32, tag="rs")
        sea = persist.tile([128, NM], dt.float32, tag="sea")
        qsum = persist.tile([128, NM], dt.float32, tag="qsum")
        outbuf = persist.tile([128, 16], dt.float32, tag="outbuf")

        # Explicit per-queue piece lists, ordered by consumption time:
        # T0(m0) = windows 0+1 both k-pairs (first 1MB) lands first so the
        # ACT stream starts ~6us earlier; the Scalar queue carries only
        # late-needed pieces (its head hosts the ACT table load).
        dma_in = {}
        _qlists = [
            (nc.sync, [(0, 0, 0), (0, 0, 1), (0, 1, 0), (0, 1, 1),
                       (1, 0, 0), (1, 1, 1), (1, 3, 0), (0, 4, 0),
                       (1, 4, 0)]),
            (nc.gpsimd, [(1, 0, 1), (1, 1, 0), (0, 2, 0), (0, 3, 0),
                         (1, 2, 0), (0, 4, 1), (1, 4, 1)]),
            (nc.scalar, [(0, 2, 1), (0, 3, 1), (1, 2, 1), (1, 3, 1)]),
        ]
        nc.scalar.dma_start(i32[:], ident_f32[:])
        for q, lst in _qlists:
            for cp, w, h in lst:
                dma_in[(cp, w, h)] = q.dma_start(
                    rnP[(cp, w, h)],
                    rnT_in[:, 2 * cp:2 * cp + 2,
                           w * 1024 + h * 512:w * 1024 + (h + 1) * 512])

        def sdep(inst, dma):
            inst.ins.add_dependency(dma.ins.name,
                                    bass_rust.DependencyInfo.SYNC_ONLY)

        # PSUM WAR edges are missing from Tile's tracker: a slot-recycling
        # matmul (start=True resets the region) must wait for the previous
        # occupant's readers (exp / diag STT). Track readers per pool slot.
        ps_readers = {}
        mv_dep_done = set()

        def war_dep(mm, readers):
            for rname in readers:
                mm.ins.add_dependency(rname, bass_rust.DependencyInfo.SYNC_ONLY)

        # The matmul MOVING-operand read has no tracked edge to the DMA
        # that writes it (only the Ldweights/stationary read is tracked);
        # add one manual edge per piece on its first reader.
        def mv_dep(mm, cp, w, h):
            if (cp, w, h) not in mv_dep_done:
                mv_dep_done.add((cp, w, h))
                sdep(mm, dma_in[(cp, w, h)])

        def stat_ap(c2, t):
            return rnP[(c2, 0, t // 4)][:, :, (t % 4) * 128:
                                        (t % 4) * 128 + 128]

        tilectr = 0
        selfexp_t = []

        def gemm_a_tile(t, last=False):
            """m-tile t.

            T0 PSUM layout: [g1 (1024) | own-triangle (1024-128t)], the
            own block computed only for cols >= t*128 (the strictly-lower
            part is recovered on the host from the own colacc region by
            symmetry).  The triangle splits into 512-aligned PSUM pieces
            so matmul outputs never straddle a bank: t<4 -> full (w0,h1)
            piece at 1024 + partial (w0,h0) at 1536; t>=4 -> partial
            (w0,h1) at 1024.  The own-diag block is the partial piece's
            head.  T1 = [+2|+3] unchanged.
            """
            nonlocal tilectr
            slot0 = tilectr % 2
            slot1 = (tilectr + 1) % 2
            ps0 = ps.tile([128, 2048], dt.float32, tag="ps")
            ps1 = ps.tile([128, 2048], dt.float32, tag="ps")
            w0 = 2048 - 128 * t       # T0 width
            if t < 4:
                tri = [(0, 128 * t, 512 - 128 * t, 1536), (1, 0, 512, 1024)]
                dcol = 1536
            else:
                tri = [(1, 128 * (t - 4), 1024 - 128 * t, 1024)]
                dcol = 1024
            for c2 in range(2):
                stat = stat_ap(c2, t)
                # T0: triangle pieces (window 0, lands first) then g1
                t0_pieces = ([(0, h, off, ln, pcol)
                              for (h, off, ln, pcol) in tri]
                             + [(1, h, 0, 512, h * 512) for h in range(2)])

                def t0_mms():
                    for pi, (w, h, off, ln, pcol) in enumerate(t0_pieces):
                        mm = nc.tensor.matmul(
                            ps0[:, pcol:pcol + ln], stat,
                            rnP[(c2, w, h)][:, :, off:off + ln],
                            start=(c2 == 0), stop=(c2 == 1), perf_mode=DR)
                        if c2 == 0 and pi == 0:
                            war_dep(mm, ps_readers.get(slot0, ()))
                        mv_dep(mm, c2, w, h)

                def t1_mms():
                    for piece in range(4):
                        w = 2 + piece // 2
                        h = piece % 2
                        mm = nc.tensor.matmul(
                            ps1[:, piece * 512:(piece + 1) * 512], stat,
                            rnP[(c2, w, h)][:, :, :],
                            start=(c2 == 0), stop=(c2 == 1), perf_mode=DR)
                        if c2 == 0 and piece == 0:
                            war_dep(mm, ps_readers.get(slot1, ()))
                        mv_dep(mm, c2, w, h)

                if last and c2 == 1:
                    # finish T1 first so its (larger) exp + raw ship
                    # overlap T0's final matmuls
                    t1_mms()
                    t0_mms()
                else:
                    t0_mms()
                    t1_mms()
            if last:
                # T1's exp + raw ship go FIRST (T1 matmuls finished first
                # above); they overlap T0's final matmuls + exp
                e1 = e1p.tile([128, 2048], dt.bfloat16, tag="e1")
                ex1 = nc.scalar.activation(e1[:], ps1[:], Act.Exp,
                                           scale=SCALE,
                                           accum_out=rs[:, 8 + t:9 + t])
                ps_readers[slot1] = [ex1.ins.name]
                nc.sync.dma_start(e1last[:], e1[:])
            # T0 exp (f32: the stored diag equals the accumulated self
            # term bit-exactly, keeping the self-cancellation exact)
            e0 = e0p.tile([128, 2048], dt.float32, tag="e0")
            ex0 = nc.scalar.activation(e0[:, 0:w0], ps0[:, 0:w0], Act.Exp,
                                       scale=SCALE,
                                       accum_out=rs[:, t:t + 1])
            ps_readers[slot0] = [ex0.ins.name]
            # self-exclusion: extract exp(self/tau) from the SBUF exp tile
            # (not PSUM - the exp is then the only PSUM reader and the PE
            # is never gated on the DVE)
            scr = scrp.tile([128, 128], dt.float32, tag="scrd")
            nc.vector.scalar_tensor_tensor(
                out=scr[:], in0=e0[:, dcol:dcol + 128], scalar=1.0,
                in1=i32[:], op0=Alu.mult, op1=Alu.mult,
                accum_out=sea[:, t:t + 1])
            # colacc block +1 (cols 1024..2048 -> colacc 0..1024)
            if t == 0:
                nc.vector.tensor_scalar(out=colacc[:, 0:1024],
                                        in0=e0[:, 0:1024], scalar1=0.0,
                                        scalar2=None, op0=Alu.add)
            else:
                nc.vector.tensor_add(colacc[:, 0:1024], colacc[:, 0:1024],
                                     e0[:, 0:1024])
            # own-triangle colacc (diag block excluded): region 4096..5120
            if t < 4:
                if t == 0:
                    nc.vector.tensor_scalar(out=colacc[:, 4608:5120],
                                            in0=e0[:, 1024:1536],
                                            scalar1=0.0, scalar2=None,
                                            op0=Alu.add)
                else:
                    nc.vector.tensor_add(colacc[:, 4608:5120],
                                         colacc[:, 4608:5120],
                                         e0[:, 1024:1536])
                ln2 = 384 - 128 * t   # partial piece minus diag block
                if ln2 > 0:
                    lo = 4224 + 128 * t
                    if t == 0:
                        nc.vector.tensor_scalar(out=colacc[:, lo:4608],
                                                in0=e0[:, 1664:1664 + ln2],
                                                scalar1=0.0, scalar2=None,
                                                op0=Alu.add)
                    else:
                        nc.vector.tensor_add(colacc[:, lo:4608],
                                             colacc[:, lo:4608],
                                             e0[:, 1664:1664 + ln2])
            else:
                ln2 = 896 - 128 * t
                if ln2 > 0:
                    lo = 4224 + 128 * t
                    nc.vector.tensor_add(colacc[:, lo:lo + ln2],
                                         colacc[:, lo:lo + ln2],
                                         e0[:, 1152:1152 + ln2])
            if last:
                # +1 and own colacc regions final
                nc.scalar.dma_start(cacc_out[:, 0:1024], colacc[:, 0:1024])
                nc.scalar.dma_start(cacc_out[:, 4096:5120],
                                    colacc[:, 4096:5120])
            else:
                # T1: bf16 exp, colacc blocks +2/+3
                e1 = e1p.tile([128, 2048], dt.bfloat16, tag="e1")
                ex1 = nc.scalar.activation(e1[:], ps1[:], Act.Exp,
                                           scale=SCALE,
                                           accum_out=rs[:, 8 + t:9 + t])
                ps_readers[slot1] = [ex1.ins.name]
                if t == 0:
                    nc.vector.tensor_scalar(out=colacc[:, 1024:3072],
                                            in0=e1[:], scalar1=0.0,
                                            scalar2=None, op0=Alu.add)
                else:
                    nc.vector.tensor_add(colacc[:, 1024:3072],
                                         colacc[:, 1024:3072], e1[:])
            tilectr += 2

        def quad_tile(qt):
            """antipodal quadrants for m-tiles 4qt..4qt+3, packed in one
            PSUM tile; row sums on the DVE, colacc region final after."""
            nonlocal tilectr
            slot = tilectr % 2
            psq = ps.tile([128, 2048], dt.float32, tag="ps")
            for c2 in range(2):
                for i in range(4):
                    t = qt * 4 + i
                    h = 0 if t < 4 else 1
                    mm = nc.tensor.matmul(
                        psq[:, i * 512:(i + 1) * 512], stat_ap(c2, t),
                        rnP[(c2, 4, h)][:, :, :],
                        start=(c2 == 0), stop=(c2 == 1), perf_mode=DR)
                    if c2 == 0 and i == 0:
                        war_dep(mm, ps_readers.get(slot, ()))
                    mv_dep(mm, c2, 4, h)
            eq = e1p.tile([128, 2048], dt.bfloat16, tag="e1")
            exq = nc.scalar.activation(eq[:], psq[:], Act.Exp, scale=SCALE)
            ps_readers[slot] = [exq.ins.name]
            # positives: post-exp quadrant diagonals (host takes log)
            for i in range(4):
                t = qt * 4 + i
                scr = scrp.tile([128, 128], dt.float32, tag="scrd")
                nc.vector.scalar_tensor_tensor(
                    out=scr[:], in0=eq[:, i * 512 + (t % 4) * 128:
                                       i * 512 + (t % 4) * 128 + 128],
                    scalar=1.0, in1=i32[:], op0=Alu.mult, op1=Alu.mult,
                    accum_out=outbuf[:, 8 + t:9 + t])
            # row sums of the 4 quads on the DVE (one op)
            nc.vector.reduce_sum(qsum[:, qt * 4:qt * 4 + 4],
                                 eq[:].rearrange("p (a w) -> p a w", a=4),
                                 axis=mybir.AxisListType.X)
            # colacc quad region 3072+qt*512 .. 3584+qt*512
            creg = slice(3072 + qt * 512, 3584 + qt * 512)
            for i in range(4):
                esub = eq[:, i * 512:(i + 1) * 512]
                if i == 0:
                    nc.vector.tensor_scalar(out=colacc[:, creg], in0=esub,
                                            scalar1=0.0, scalar2=None,
                                            op0=Alu.add)
                else:
                    nc.vector.tensor_add(colacc[:, creg], colacc[:, creg],
                                         esub)
            tilectr += 1
            # quad colacc region is final: stream it out early
            nc.gpsimd.dma_start(cacc_out[:, creg], colacc[:, creg])

        # Quad phases interleave mid-GEMM so their exp/colacc/reduce work
        # and output DMAs overlap GEMM-A instead of forming a tail.
        for t in (0, 1, 2, 3):
            gemm_a_tile(t)
        quad_tile(0)
        for t in (4, 5, 6):
            gemm_a_tile(t)
        # +2/+3 colacc region is final after m6 (m7's T1 ships raw);
        # sync queue is idle mid-stream
        nc.sync.dma_start(cacc_out[:, 1024:2048], colacc[:, 1024:2048])
        nc.sync.dma_start(cacc_out[:, 2048:3072], colacc[:, 2048:3072])
        quad_tile(1)
        gemm_a_tile(7, last=True)

        # ---- epilogue ----
        rsum = smallp.tile([128, NM], dt.float32, tag="rsum")
        nc.vector.tensor_add(rsum[:], rs[:, 0:8], rs[:, 8:16])
        nc.vector.tensor_add(rsum[:], rsum[:], qsum[:])
        nc.vector.tensor_sub(outbuf[:, 0:8], rsum[:], sea[:])
        nc.sync.dma_start(out[:], outbuf[:])

    # Pin bacc's activation-table choice to the one table holding Exp (and
    # Ln/Copy) so exactly one ACT table load is emitted.
    import concourse.bacc as bacc_mod
    _orig_tables = bacc_mod.get_activation_tables

    def _only_lnexp(arch):
        keep = "natural_log_exp_and_others"
        return {k: (v if k == keep else set())
                for k, v in _orig_tables(arch).items()}

    bacc_mod.get_activation_tables = _only_lnexp
    try:
        nc.compile()
    finally:
        bacc_mod.get_activation_tables = _orig_tables
    _prune_redundant_ldweights(nc)
    return nc


def _prune_redundant_ldweights(nc):
    """Drop InstLdweights that reload the stationary already resident in
    the PE array (bass emits one per matmul; the array keeps weights
    across matmuls, so only the first load of each group is needed).
    Only sync-free loads (no semaphore waits/updates) with a weights AP
    identical to the previously kept load are removed: ~100 of 136 go,
    each ~150-230ns of PE pipeline."""
    for f in nc.m.functions:
        for b in f.blocks:
            keep = []
            last_sig = None
            changed = False
            for ins in b.instructions:
                tn = type(ins).__name__
                if tn == 'InstLdweights':
                    sig = (str(ins.ins[0]), str(ins.perf_mode),
                           str(ins.is_transpose))
                    clean = (ins.sync_info is None
                             and not ins.has_wait()
                             and not ins.has_update())
                    if clean and sig == last_sig:
                        changed = True
                        continue
                    last_sig = sig
                keep.append(ins)
            if changed:
                b.instructions = keep


def _col_rows(c):
    """Global row indices of core c's 5120 GEMM columns, in rnT order."""
    b = (c + 4) % NCORES
    idxs = [np.arange(((c + d) % NCORES) * RPC, ((c + d) % NCORES + 1) * RPC)
            for d in range(4)]
    if c < 4:
        q = np.arange(b * RPC, (b + 1) * RPC)
    else:
        q = np.concatenate([np.arange(b * RPC + 512, (b + 1) * RPC),
                            np.arange(b * RPC, b * RPC + 512)])
    idxs.append(q)
    return np.concatenate(idxs)


def _host_inputs(zi, zj):
    reps = np.concatenate([np.asarray(zi, np.float64),
                           np.asarray(zj, np.float64)], axis=0)
    norms = np.maximum(np.linalg.norm(reps, axis=1, keepdims=True), 1e-8)
    rn8 = (FP8S * reps / norms).astype(np.float32).astype(
        ml_dtypes.float8_e4m3)                              # [8192, 512]
    ident_f32 = np.eye(128, dtype=np.float32)
    in_maps = []
    for c in range(NCORES):
        xt = rn8[_col_rows(c)].T                            # [512, 5120]
        rnT = np.ascontiguousarray(
            xt.reshape(KC, 128, CTOT).transpose(1, 0, 2))   # [128, 4, 5120]
        in_maps.append({"rnT": rnT, "ident_f32": ident_f32})
    return in_maps


def _postprocess(results):
    denom = np.zeros(ROWS, np.float64)
    pos = np.zeros(ROWS, np.float64)
    for c in range(NCORES):
        o = np.asarray(results[c]["out"], np.float64)        # [128, 16]
        ca = np.asarray(results[c]["cacc_out"], np.float64)  # [128, 5120]
        e1l = np.asarray(results[c]["e1last"], np.float64)   # [128, 2048]
        cr = _col_rows(c)
        for t in range(NM):
            rows = slice(c * RPC + t * 128, c * RPC + (t + 1) * 128)
            denom[rows] += o[:, t]
        # colsum partials: fold partitions, scatter to owning rows.
        # [0:3072] = blocks +1/+2/+3 (m7's +2/+3 tile shipped raw in
        # e1last), [3072:4096] = antipodal quadrants, [4096:5120] = the
        # own-block triangle (strictly-lower part of own rows by
        # symmetry; rows 0..128 of the block have no lower part).
        colsum = ca.sum(axis=0)                              # [5120]
        np.add.at(denom, cr[1024:4096], colsum[0:3072])
        np.add.at(denom, cr[4096:5120], colsum[3072:4096])
        np.add.at(denom, cr[2048:4096], e1l.sum(axis=0))
        np.add.at(denom, cr[128:1024], colsum[4224:5120])
        if c < 4:
            # o[:, 8:16] = exp(pos/tau) (post-exp diag extraction)
            opos = o[:, 8:16].T.reshape(-1)                  # [1024]
            rows = np.arange(c * RPC, (c + 1) * RPC)
            pos[rows] = opos
            pos[cr[4096:]] = opos
    loss = np.mean(-np.log(pos) + np.log(denom))
    return np.asarray(loss, dtype=np.float32)


def kernel(zi, zj, _trace=False):
    from concourse.bass_utils import run_bass_kernel_spmd

    if "nc" not in _prog_cache:
        _prog_cache["nc"] = _build_program()
    nc = _prog_cache["nc"]
    in_maps = _host_inputs(zi, zj)
    res = run_bass_kernel_spmd(nc, in_maps, list(range(NCORES)),
                               trace=_trace)
    _prog_cache["last_result"] = res
    return _postprocess(res.results)
